# revision 41
# baseline (speedup 1.0000x reference)
"""Trainium2 Bass kernel for the unsupervised-entropy loss.

intra = mean_r H_r where H_r = entropy(softmax(-d2(x_r, m))).
Softmax is shift-invariant, so with unit-norm m rows the logits reduce to
z = 2 x m^T (the ||x||^2 and ||m||^2 terms drop).  Per row:
  S = sum_j exp(z_j),  W = sum_j z_j exp(z_j),  H = log S - W/S

Row-to-partition assignment is contiguous (partition p owns rows
[p*256, (p+1)*256) of the shard) so every DMA chunk is one contiguous
8 KiB read per partition (128 descriptors per chunk, minimal SWDGE cost).
Row order is irrelevant: only sums over all rows are needed.

Software-pipelined device loop, per iteration i (block b = 1024 rows):
  PE : z-matmuls(b=i-1)  [col-tiled pair: chunk A -> psum partitions 0:64,
       chunk B -> 64:128, one accumulation group, concurrent col-groups]
       8 PE transposes(b=i)   x_nat [128p,128d] -> psum xT [128d,128p]
       2 reduce matmuls(b=i-2): indicator lhsT [128,16] accumulating
       S and W into one psum bank [16,512] shared by 4 blocks
  ACT: exp(psZ(i-1)) -> E bf16; evict xT tiles 0:3 psum->sbuf
  DVE: P = z*E (b=i-2); evict xT tiles 3:8; nothing else
  ACT: every 4th block: evict the [16,512] S/W bank -> bf16 staging
  GpSimd: one SWDGE cast-load (f32->bf16) of a 1 MiB chunk every 2 blocks

Final: two SBUF->SBUF DMAs fan the staged [8,8,512] S and W values onto
[128,256]; ACT Ln(+accum lsum), ACT exp(-lnS)=1/S, DVE W*rS(+accum wsum).
Output [128,2] per-partition partial sums; host reduces and adds the
(tiny) inter term.
"""

import json

import numpy as np
import ml_dtypes

import concourse.bass as _bass
import concourse.tile as _tile
from concourse import mybir
from concourse.bass_utils import run_bass_kernel_spmd
from concourse.vector_clock import ScopedClock

F32 = mybir.dt.float32
BF16 = mybir.dt.bfloat16
N, D, K = 262144, 128, 64
NCORES = 8
NSHARD = N // NCORES          # 32768 rows per core
BLK = 1024                    # rows per block
NBLK = NSHARD // BLK          # 32 blocks
NCHUNK = 16                   # DMA chunks (2 blocks each)
RPP = NSHARD // 128           # rows per partition (256)
EPS = 1e-16
LAMB = 1.0


# ---- workarounds: this walrus build rejects >1 sync wait per instruction ----

def _split_multiwait(json_bytes: bytes) -> bytes:
    data = json.loads(json_bytes)
    counter = [0]
    for fn in data["functions"]:
        for blk in fn["blocks"]:
            new_insts = []
            for inst in blk["instructions"]:
                si = inst.get("sync_info")
                waits = (si or {}).get("on_wait") or []
                if len(waits) > 1:
                    for w in waits[:-1]:
                        counter[0] += 1
                        new_insts.append({
                            "debug": inst.get("debug"),
                            "engine": inst["engine"],
                            "ins": [],
                            "name": f"splitw_{counter[0]}_{inst['name']}",
                            "opcode": "EventSemaphore",
                            "outs": [],
                            "sync_info": {"on_update": [], "on_wait": [w]},
                        })
                    si["on_wait"] = [waits[-1]]
                new_insts.append(inst)
            blk["instructions"] = new_insts
    return json.dumps(data).encode()


class PatchedBass(_bass.Bass):
    def to_json_bytes(self) -> bytes:
        return _split_multiwait(super().to_json_bytes())


class SplitDrainTileContext(_tile.TileContext):
    def _drain_and_barrier(self, tick_clock, wait_clock):
        drain_inst = self.nc.sync.drain()
        wait_clock.add_sem_waits(
            drain_inst.ins, ScopedClock({None: tick_clock.global_clock})
        )
        si = drain_inst.ins.sync_info
        if si is not None and len(si.on_wait) > 1:
            waits = list(si.on_wait)
            si.on_wait = waits[:1]
            drain_inst.ins.sync_info = si
            for w in waits[1:]:
                d2 = self.nc.sync.drain()
                si2 = d2.ins.sync_info
                if si2 is None:
                    import copy
                    si2 = copy.copy(si)
                si2.on_wait = [w]
                si2.on_update = []
                d2.ins.sync_info = si2
        self.nc.all_engine_barrier()
        assert self.sems is not None
        popped = self.nc._tile_sem_poison_stack.pop()
        assert popped is self._sem_poison
        self.nc.clear_and_free_semaphores(list(self.sems.allocated().values()))
        self.nc.all_engine_barrier()


# ------------------------------ kernel build ------------------------------

_CACHE = {}


def _build():
    if "nc" in _CACHE:
        return _CACHE["nc"]
    nc = PatchedBass("TRN2", target_bir_lowering=False, debug=False)
    xs_ap = nc.dram_tensor("xs", [NSHARD, D], F32, kind="ExternalInput").ap()
    m2t_ap = nc.dram_tensor("m2t", [D, K], BF16, kind="ExternalInput").ap()
    ind_ap = nc.dram_tensor("ind", [128, 8, 8], BF16, kind="ExternalInput").ap()
    id_ap = nc.dram_tensor("ident", [128, 128], BF16, kind="ExternalInput").ap()
    out_ap = nc.dram_tensor("out", [128, 16], F32, kind="ExternalOutput").ap()

    Exp = mybir.ActivationFunctionType.Exp
    Ln = mybir.ActivationFunctionType.Ln
    MUL = mybir.AluOpType.mult

    # row = p*(NCHUNK*16) + c*16 + r: partition p owns a contiguous row
    # range; each 2-block chunk load is one contiguous 8 KiB per partition.
    xsv = xs_ap.rearrange("(p c r) d -> c p r d", p=128, c=NCHUNK)

    with SplitDrainTileContext(nc) as tc:
        with tc.tile_pool(name="const", bufs=1) as const, \
             tc.tile_pool(name="xin", bufs=4) as xin, \
             tc.tile_pool(name="xtp", bufs=3) as xtp, \
             tc.tile_pool(name="ep", bufs=3) as ep, \
             tc.tile_pool(name="pp", bufs=3) as pp, \
             tc.tile_pool(name="stage", bufs=1) as stage, \
             tc.tile_pool(name="fin", bufs=1) as fin, \
             tc.tile_pool(name="psT", bufs=2, space="PSUM") as psTp, \
             tc.tile_pool(name="psZ", bufs=2, space="PSUM") as psZp, \
             tc.tile_pool(name="psSW", bufs=1, space="PSUM") as psSWp, \
             tc.tile_pool(name="psF", bufs=1, space="PSUM") as psFp:

            m2t = const.tile([D, K], BF16)
            nc.sync.dma_start(out=m2t[:], in_=m2t_ap[:])
            ind = const.tile([128, 8, 8], BF16)
            nc.sync.dma_start(out=ind[:], in_=ind_ap[:])
            ident = const.tile([128, 128], BF16)
            nc.sync.dma_start(out=ident[:], in_=id_ap[:])

            # staged S/W per 4-block group (rows 0:8 = S, 32:40 = W,
            # rows 8:32 are dead padding so one evict instruction covers
            # both at the same per-partition free-dim cost)
            stats_sw = stage.tile([40, 8, 512], BF16)
            # per-group partial sums: cols 0:8 = sum ln S, 8:16 = sum W/S
            lsw = fin.tile([128, 16], F32)
            # per-group scratch, allocated once and reused (fewer tiles =
            # fewer release sems = shorter end-of-kernel sem-clear tail)
            psF = psFp.tile([128, 4, 40], BF16)
            psSW = psSWp.tile([40, 512], F32)
            lnSg = fin.tile([128, 4, 8], F32)
            rSg = fin.tile([128, 4, 8], F32)
            scrg = fin.tile([128, 4, 8], F32)

            xc_t = {}     # chunk -> xin tile
            xT_t = {}     # pair -> xtp tile (sbuf xT, 2 blocks)
            psZ_t = {}    # pair -> psum z tile
            E_t = {}      # pair -> E tile
            P_t = {}      # pair -> P tile

            def fan_transposes(g):
                # fan group g's staged [40,512] stats across 128 partitions
                # with 4 tiny PE transposes into one PSUM bank (no DMA).
                for c in range(4):
                    nc.tensor.transpose(psF[:, c, :],
                                        stats_sw[:, g, 128 * c:128 * (c + 1)],
                                        ident[0:40, 0:40])

            def group_final(g):
                sfan = psF[:, :, 0:8]       # [128, 4, 8]
                wfan = psF[:, :, 32:40]
                nc.scalar.activation(lnSg[:], sfan, Ln,
                                     accum_out=lsw[:, g:g + 1])
                nc.scalar.activation(rSg[:], lnSg[:], Exp, scale=-1.0)
                nc.vector.scalar_tensor_tensor(
                    scrg[:], wfan, 1.0, rSg[:], MUL, MUL,
                    accum_out=lsw[:, 8 + g:9 + g])

            for i in range(NBLK + 7):
                # ---- GpSimd: load 2-block chunk (prefetch via pool) ----
                if i < NBLK and i % 2 == 0:
                    c = i // 2
                    xc = xin.tile([128, 16, D], BF16)
                    xc_t[c] = xc
                    nc.gpsimd.dma_start(out=xc[:], in_=xsv[c])

                # ---- PE: z-matmuls for block i-2 (xT evicted 2 iters ago).
                # Even/odd blocks share a 2-bank psZ pair tile so the exp
                # and z*E passes amortize their fixed costs over 2 blocks.
                if 2 <= i <= NBLK + 1:
                    b = i - 2
                    q, h = divmod(b, 2)
                    xTf = xT_t[q][:, 8 * h:8 * h + 8, :].rearrange(
                        "d r p -> d (r p)")
                    if h == 0:
                        psZ_t[q] = psZp.tile([128, 2, 512], F32, name="psZ")
                    psZ = psZ_t[q]
                    nc.tensor.matmul(psZ[0:64, h, :], m2t[:], xTf[:, 0:512],
                                     start=True, stop=True,
                                     tile_position=(0, 0))
                    nc.tensor.matmul(psZ[64:128, h, :], m2t[:],
                                     xTf[:, 512:1024],
                                     start=True, stop=True,
                                     tile_position=(0, 64))

                # ---- ACT: exp for block pair (i-3, i-2) ----
                if 2 <= i <= NBLK + 1 and (i - 2) % 2 == 1:
                    q = (i - 2) // 2
                    E = ep.tile([128, 2, 512], BF16)
                    E_t[q] = E
                    nc.scalar.activation(E[:], psZ_t[q][:], Exp)

                # ---- DVE: P = z*E for block pair (i-4, i-3) ----
                if 3 <= i <= NBLK + 2 and (i - 3) % 2 == 1:
                    q3 = (i - 3) // 2
                    P = pp.tile([128, 2, 512], BF16)
                    P_t[q3] = P
                    nc.vector.scalar_tensor_tensor(P[:], psZ_t[q3][:], 1.0,
                                                   E_t[q3][:], MUL, MUL)

                # ---- PE: 8 transposes for block i ----
                if i < NBLK:
                    c, h = divmod(i, 2)
                    xc = xc_t[c]
                    psT = psTp.tile([128, 8, 128], BF16)
                    for r in range(8):
                        nc.tensor.transpose(psT[:, r, :], xc[:, 8 * h + r, :],
                                            ident[:])
                    if h == 0:
                        xT_t[c] = xtp.tile([128, 16, 128], BF16, name="xT")
                    # all 8 tiles on DVE: its bf16 2x copy beats splitting
                    # (ACT's ~300ns per-instruction fixed cost dominates)
                    nc.vector.tensor_copy(xT_t[c][:, 8 * h:8 * h + 8, :],
                                          psT[:])

                # ---- PE: reduce matmuls for block i-5 (S || W col groups) ----
                if 5 <= i <= NBLK + 4:
                    b5 = i - 5
                    g, j = divmod(b5, 4)
                    # S group occupies array cols 0:8 -> psum rows 0:8,
                    # W group cols 32:40 -> rows 32:40; distinct col groups
                    # run concurrently on the PE. j==0 matmuls write each
                    # region fully (zeros outside their 2 rows), so later
                    # start=False matmuls accumulate onto clean zeros.
                    q5, h5 = divmod(b5, 2)
                    nc.tensor.matmul(psSW[0:8, :], ind[:, j, :],
                                     E_t[q5][:, h5, :],
                                     start=(j == 0), stop=(j == 3),
                                     tile_position=(0, 0),
                                     skip_group_check=True)
                    nc.tensor.matmul(psSW[32:40, :], ind[:, 4 + j, :],
                                     P_t[q5][:, h5, :],
                                     start=(j == 0), stop=(j == 3),
                                     tile_position=(0, 32),
                                     skip_group_check=True)
                    if j == 3:
                        nc.scalar.copy(stats_sw[:, g, :], psSW[:])

                # staggered one iteration after the group's swevict so the
                # PE stream never head-of-line blocks on ACT
                if i >= 9 and (i - 9) % 4 == 0 and (i - 9) // 4 < 8:
                    fan_transposes((i - 9) // 4)
                if i >= 10 and (i - 10) % 4 == 0 and (i - 10) // 4 < 8:
                    group_final((i - 10) // 4)

                # free refs we no longer need (python-side bookkeeping only)
                if i >= 8:
                    qold = (i - 8) // 2
                    xT_t.pop(qold, None)
                    psZ_t.pop(qold, None)
                    E_t.pop(qold, None)
                    P_t.pop(qold, None)

            nc.sync.dma_start(out=out_ap[:], in_=lsw[:])

    _CACHE["nc"] = nc
    return nc


def _entropy_np(p):
    p = np.where(p <= 0, EPS, p)
    p = np.where(p >= 1, 1.0 - EPS, p)
    return -np.sum(p * np.log(p), axis=-1)


def kernel(x, m):
    nc = _build()

    m2t = (2.0 * np.float64(m).T).astype(ml_dtypes.bfloat16)   # [128, 64]
    ident = np.eye(128, dtype=ml_dtypes.bfloat16)
    ind = np.zeros((128, 8, 8), dtype=ml_dtypes.bfloat16)
    for j in range(4):
        ind[0:64, j, 2 * j] = 1          # S, chunk A (psum rows 0:8)
        ind[64:128, j, 2 * j + 1] = 1    # S, chunk B
        ind[0:64, 4 + j, 2 * j] = 1      # W, chunk A (psum rows 32:40)
        ind[64:128, 4 + j, 2 * j + 1] = 1

    in_maps = []
    for c in range(NCORES):
        in_maps.append({
            "xs": np.ascontiguousarray(x[c * NSHARD:(c + 1) * NSHARD]),
            "m2t": m2t, "ind": ind, "ident": ident,
        })
    _CACHE["last_in_maps"] = in_maps
    res = run_bass_kernel_spmd(nc, in_maps, core_ids=list(range(NCORES)))

    tot_ls = 0.0
    tot_ws = 0.0
    for c in range(NCORES):
        o = np.float64(res.results[c]["out"])
        tot_ls += o[:, 0:8].sum()
        tot_ws += o[:, 8:16].sum()
    intra = (tot_ls - tot_ws) / N

    # inter term on host (tiny), replicating the reference exactly
    m64 = np.float64(m)
    mu = m64.mean(axis=0)
    d2 = ((mu[None, :] - m64) ** 2).sum(axis=1)
    zl = -d2
    zl -= zl.max()
    e = np.exp(zl)
    p = e / e.sum()
    inter = _entropy_np(p)

    total = intra - LAMB * inter
    return (np.float32(total), np.float32(intra), np.float32(inter))


# revision 44
# speedup vs baseline: 1.1404x; 1.1404x over previous
"""Trainium2 Bass kernel for the unsupervised-entropy loss.

intra = mean_r H_r where H_r = entropy(softmax(-d2(x_r, m))).
Softmax is shift-invariant, so with unit-norm m rows the logits reduce to
z = 2 x m^T (the ||x||^2 and ||m||^2 terms drop).  Per row:
  S = sum_j exp(z_j),  W = sum_j z_j exp(z_j),  H = log S - W/S

Row-to-partition assignment is contiguous (partition p owns rows
[p*256, (p+1)*256) of the shard) so every DMA chunk is one contiguous
8 KiB read per partition (128 descriptors per chunk, minimal SWDGE cost).
Row order is irrelevant: only sums over all rows are needed.

Software-pipelined device loop, per iteration i (block b = 1024 rows):
  PE : z-matmuls(b=i-1)  [col-tiled pair: chunk A -> psum partitions 0:64,
       chunk B -> 64:128, one accumulation group, concurrent col-groups]
       8 PE transposes(b=i)   x_nat [128p,128d] -> psum xT [128d,128p]
       2 reduce matmuls(b=i-2): indicator lhsT [128,16] accumulating
       S and W into one psum bank [16,512] shared by 4 blocks
  ACT: exp(psZ(i-1)) -> E bf16; evict xT tiles 0:3 psum->sbuf
  DVE: P = z*E (b=i-2); evict xT tiles 3:8; nothing else
  ACT: every 4th block: evict the [16,512] S/W bank -> bf16 staging
  GpSimd: one SWDGE cast-load (f32->bf16) of a 1 MiB chunk every 2 blocks

Final: two SBUF->SBUF DMAs fan the staged [8,8,512] S and W values onto
[128,256]; ACT Ln(+accum lsum), ACT exp(-lnS)=1/S, DVE W*rS(+accum wsum).
Output [128,2] per-partition partial sums; host reduces and adds the
(tiny) inter term.
"""

import json

import numpy as np
import ml_dtypes

import concourse.bass as _bass
import concourse.tile as _tile
from concourse import mybir
from concourse.bass_utils import run_bass_kernel_spmd
from concourse.vector_clock import ScopedClock

F32 = mybir.dt.float32
BF16 = mybir.dt.bfloat16
N, D, K = 262144, 128, 64
NCORES = 8
NSHARD = N // NCORES          # 32768 rows per core
BLK = 1024                    # rows per block
NBLK = NSHARD // BLK          # 32 blocks
NCHUNK = 16                   # DMA chunks (2 blocks each)
RPP = NSHARD // 128           # rows per partition (256)
EPS = 1e-16
LAMB = 1.0


# ---- workarounds: this walrus build rejects >1 sync wait per instruction ----

def _split_multiwait(json_bytes: bytes) -> bytes:
    data = json.loads(json_bytes)
    counter = [0]
    for fn in data["functions"]:
        for blk in fn["blocks"]:
            new_insts = []
            for inst in blk["instructions"]:
                si = inst.get("sync_info")
                waits = (si or {}).get("on_wait") or []
                if len(waits) > 1:
                    for w in waits[:-1]:
                        counter[0] += 1
                        new_insts.append({
                            "debug": inst.get("debug"),
                            "engine": inst["engine"],
                            "ins": [],
                            "name": f"splitw_{counter[0]}_{inst['name']}",
                            "opcode": "EventSemaphore",
                            "outs": [],
                            "sync_info": {"on_update": [], "on_wait": [w]},
                        })
                    si["on_wait"] = [waits[-1]]
                new_insts.append(inst)
            blk["instructions"] = new_insts
    return json.dumps(data).encode()


class PatchedBass(_bass.Bass):
    def to_json_bytes(self) -> bytes:
        return _split_multiwait(super().to_json_bytes())


class SplitDrainTileContext(_tile.TileContext):
    def _drain_and_barrier(self, tick_clock, wait_clock):
        drain_inst = self.nc.sync.drain()
        wait_clock.add_sem_waits(
            drain_inst.ins, ScopedClock({None: tick_clock.global_clock})
        )
        si = drain_inst.ins.sync_info
        if si is not None and len(si.on_wait) > 1:
            waits = list(si.on_wait)
            si.on_wait = waits[:1]
            drain_inst.ins.sync_info = si
            for w in waits[1:]:
                d2 = self.nc.sync.drain()
                si2 = d2.ins.sync_info
                if si2 is None:
                    import copy
                    si2 = copy.copy(si)
                si2.on_wait = [w]
                si2.on_update = []
                d2.ins.sync_info = si2
        self.nc.all_engine_barrier()
        assert self.sems is not None
        popped = self.nc._tile_sem_poison_stack.pop()
        assert popped is self._sem_poison
        self.nc.clear_and_free_semaphores(list(self.sems.allocated().values()))
        self.nc.all_engine_barrier()


# ------------------------------ kernel build ------------------------------

_CACHE = {}


def _build():
    if "nc" in _CACHE:
        return _CACHE["nc"]
    nc = PatchedBass("TRN2", target_bir_lowering=False, debug=False)
    xs_ap = nc.dram_tensor("xs", [NSHARD, D], F32, kind="ExternalInput").ap()
    m2t_ap = nc.dram_tensor("m2t", [D, K], BF16, kind="ExternalInput").ap()
    ind_ap = nc.dram_tensor("ind", [128, 8, 8], BF16, kind="ExternalInput").ap()
    id_ap = nc.dram_tensor("ident", [128, 128], BF16, kind="ExternalInput").ap()
    out_ap = nc.dram_tensor("out", [40, 8, 512], BF16,
                            kind="ExternalOutput").ap()

    Exp = mybir.ActivationFunctionType.Exp
    Ln = mybir.ActivationFunctionType.Ln
    MUL = mybir.AluOpType.mult

    # row = p*(NBLK*8) + b*8 + r: partition p owns a contiguous row range,
    # so each per-block load is one contiguous 4 KiB read per partition.
    xsv = xs_ap.rearrange("(p b r) d -> b p r d", p=128, b=NBLK)

    with SplitDrainTileContext(nc) as tc:
        with tc.tile_pool(name="const", bufs=1) as const, \
             tc.tile_pool(name="xin", bufs=8) as xin, \
             tc.tile_pool(name="xtp", bufs=4) as xtp, \
             tc.tile_pool(name="ep", bufs=5) as ep, \
             tc.tile_pool(name="pp", bufs=4) as pp, \
             tc.tile_pool(name="stage", bufs=1) as stage, \
             tc.tile_pool(name="psT", bufs=2, space="PSUM") as psTp, \
             tc.tile_pool(name="psZ", bufs=2, space="PSUM") as psZp, \
             tc.tile_pool(name="psSW", bufs=2, space="PSUM") as psSWp:

            m2t = const.tile([D, K], BF16)
            nc.sync.dma_start(out=m2t[:], in_=m2t_ap[:])
            ind = const.tile([128, 8, 8], BF16)
            nc.sync.dma_start(out=ind[:], in_=ind_ap[:])
            ident = const.tile([128, 128], BF16)
            nc.sync.dma_start(out=ident[:], in_=id_ap[:])

            # staged S/W per 4-block group (rows 0:8 = S, 32:40 = W,
            # rows 8:32 are dead padding so one evict instruction covers
            # both at the same per-partition free-dim cost)
            stats_sw = stage.tile([40, 8, 512], BF16)
            xc_t = {}     # block -> xin tile
            xT_t = {}     # block -> xtp tile (sbuf xT)
            psZ_t = {}    # pair -> psum z tile
            E_t = {}      # pair -> E tile
            P_t = {}      # pair -> P tile
            psSW_t = {}   # group -> psum stats tile

            for i in range(NBLK + 5):
                # ---- GpSimd: load block i (prefetch governed by pool) ----
                if i < NBLK:
                    xc = xin.tile([128, 8, D], BF16)
                    xc_t[i] = xc
                    nc.gpsimd.dma_start(out=xc[:], in_=xsv[i])

                # ---- PE: z-matmuls for block i-2 (xT evicted 2 iters ago).
                # Even/odd blocks share a 2-bank psZ pair tile so the exp
                # and z*E passes amortize their fixed costs over 2 blocks.
                if 2 <= i <= NBLK + 1:
                    b = i - 2
                    q, h = divmod(b, 2)
                    xTf = xT_t[b][:].rearrange("d r p -> d (r p)")
                    if h == 0:
                        psZ_t[q] = psZp.tile([128, 2, 512], F32, name="psZ")
                    psZ = psZ_t[q]
                    nc.tensor.matmul(psZ[0:64, h, :], m2t[:], xTf[:, 0:512],
                                     start=True, stop=True,
                                     tile_position=(0, 0))
                    nc.tensor.matmul(psZ[64:128, h, :], m2t[:],
                                     xTf[:, 512:1024],
                                     start=True, stop=True,
                                     tile_position=(0, 64))

                # ---- ACT: exp for block pair (i-3, i-2) ----
                if 2 <= i <= NBLK + 1 and (i - 2) % 2 == 1:
                    q = (i - 2) // 2
                    E = ep.tile([128, 2, 512], BF16)
                    E_t[q] = E
                    nc.scalar.activation(E[:], psZ_t[q][:], Exp)

                # ---- DVE: P = z*E for block pair (i-4, i-3) ----
                if 3 <= i <= NBLK + 2 and (i - 3) % 2 == 1:
                    q3 = (i - 3) // 2
                    P = pp.tile([128, 2, 512], BF16)
                    P_t[q3] = P
                    nc.vector.scalar_tensor_tensor(P[:], psZ_t[q3][:], 1.0,
                                                   E_t[q3][:], MUL, MUL)

                # ---- PE: 8 transposes for block i ----
                if i < NBLK:
                    xc = xc_t[i]
                    psT = psTp.tile([128, 8, 128], BF16)
                    for r in range(8):
                        nc.tensor.transpose(psT[:, r, :], xc[:, r, :],
                                            ident[:])
                    xT = xtp.tile([128, 8, 128], BF16)
                    xT_t[i] = xT
                    # all 8 tiles on DVE: its bf16 2x copy beats splitting
                    # (ACT's ~300ns per-instruction fixed cost dominates)
                    nc.vector.tensor_copy(xT[:], psT[:])

                # ---- PE: reduce matmuls for block i-5 (S || W col groups) ----
                if 5 <= i <= NBLK + 4:
                    b5 = i - 5
                    g, j = divmod(b5, 4)
                    if j == 0:
                        psSW_t[g] = psSWp.tile([40, 512], F32, name="psSW")
                    psSW = psSW_t[g]
                    # S group occupies array cols 0:8 -> psum rows 0:8,
                    # W group cols 32:40 -> rows 32:40; distinct col groups
                    # run concurrently on the PE. j==0 matmuls write each
                    # region fully (zeros outside their 2 rows), so later
                    # start=False matmuls accumulate onto clean zeros.
                    q5, h5 = divmod(b5, 2)
                    nc.tensor.matmul(psSW[0:8, :], ind[:, j, :],
                                     E_t[q5][:, h5, :],
                                     start=(j == 0), stop=(j == 3),
                                     tile_position=(0, 0),
                                     skip_group_check=True)
                    nc.tensor.matmul(psSW[32:40, :], ind[:, 4 + j, :],
                                     P_t[q5][:, h5, :],
                                     start=(j == 0), stop=(j == 3),
                                     tile_position=(0, 32),
                                     skip_group_check=True)
                    if j == 3:
                        nc.scalar.copy(stats_sw[:, g, :], psSW[:])
                        # ship this group's raw S/W stats to DRAM now; the
                        # tiny ln/divide final runs on the host in f64
                        nc.sync.dma_start(out=out_ap[:, g, :],
                                          in_=stats_sw[:, g, :])

                # free refs we no longer need (python-side bookkeeping only)
                if i >= 8:
                    xT_t.pop(i - 8, None)
                    qold = (i - 8) // 2
                    psZ_t.pop(qold, None)
                    E_t.pop(qold, None)
                    P_t.pop(qold, None)


    _CACHE["nc"] = nc
    return nc


def _entropy_np(p):
    p = np.where(p <= 0, EPS, p)
    p = np.where(p >= 1, 1.0 - EPS, p)
    return -np.sum(p * np.log(p), axis=-1)


def kernel(x, m):
    nc = _build()

    m2t = (2.0 * np.float64(m).T).astype(ml_dtypes.bfloat16)   # [128, 64]
    ident = np.eye(128, dtype=ml_dtypes.bfloat16)
    ind = np.zeros((128, 8, 8), dtype=ml_dtypes.bfloat16)
    for j in range(4):
        ind[0:64, j, 2 * j] = 1          # S, chunk A (psum rows 0:8)
        ind[64:128, j, 2 * j + 1] = 1    # S, chunk B
        ind[0:64, 4 + j, 2 * j] = 1      # W, chunk A (psum rows 32:40)
        ind[64:128, 4 + j, 2 * j + 1] = 1

    in_maps = []
    for c in range(NCORES):
        in_maps.append({
            "xs": np.ascontiguousarray(x[c * NSHARD:(c + 1) * NSHARD]),
            "m2t": m2t, "ind": ind, "ident": ident,
        })
    _CACHE["last_in_maps"] = in_maps
    res = run_bass_kernel_spmd(nc, in_maps, core_ids=list(range(NCORES)))

    tot_ls = 0.0
    tot_ws = 0.0
    for c in range(NCORES):
        o = np.float64(res.results[c]["out"])   # [40, 8, 512] staged S/W
        S = o[0:8]
        W = o[32:40]
        tot_ls += np.log(S).sum()
        tot_ws += (W / S).sum()
    intra = (tot_ls - tot_ws) / N

    # inter term on host (tiny), replicating the reference exactly
    m64 = np.float64(m)
    mu = m64.mean(axis=0)
    d2 = ((mu[None, :] - m64) ** 2).sum(axis=1)
    zl = -d2
    zl -= zl.max()
    e = np.exp(zl)
    p = e / e.sum()
    inter = _entropy_np(p)

    total = intra - LAMB * inter
    return (np.float32(total), np.float32(intra), np.float32(inter))


# revision 45
# speedup vs baseline: 1.1474x; 1.0062x over previous
"""Trainium2 Bass kernel for the unsupervised-entropy loss.

intra = mean_r H_r where H_r = entropy(softmax(-d2(x_r, m))).
Softmax is shift-invariant, so with unit-norm m rows the logits reduce to
z = 2 x m^T (the ||x||^2 and ||m||^2 terms drop).  Per row:
  S = sum_j exp(z_j),  W = sum_j z_j exp(z_j),  H = log S - W/S

Row-to-partition assignment is contiguous (partition p owns rows
[p*256, (p+1)*256) of the shard) so every per-block load is one
contiguous 4 KiB read per partition (128 descriptors, minimal SWDGE
cost). Row order is irrelevant: only sums over all rows are needed.

Software-pipelined device loop, iteration i (block = 1024 rows; even/odd
blocks share a 2-bank psZ pair tile so exp and z*E amortize their fixed
costs over 2 blocks):
  GpSimd: SWDGE cast-load (f32->bf16) of block i, 8-deep prefetch
  PE : z-matmuls(block i-2)  [col-tiled concurrent pair via tile_position
       (0,0)/(0,64), rhs = xT from sbuf]
       8 PE transposes(block i) x_nat [128p,128d] -> psum xT [128d,128p]
       2 reduce matmuls(block i-5): indicator lhsT accumulating S into
       psum rows 0:8 and W into rows 32:40 (concurrent col groups) of a
       [40,512] bank shared by 4 blocks
  ACT: exp(psZ pair) -> E bf16 once per 2 blocks
  DVE: P = z*E (pair, once per 2 blocks); evict all 8 xT tiles (2x bf16)
  ACT: every 4th block: evict the [40,512] S/W bank -> bf16 staging
  Sync: every 4th block: DMA the group's staged stats to DRAM

Output: raw per-row S and W sums ([40, 8, 512] bf16; rows 0:8 = S,
32:40 = W). The host computes sum(ln S) - sum(W/S) in f64 and adds the
(tiny) inter term.
"""

import json

import numpy as np
import ml_dtypes

import concourse.bass as _bass
import concourse.tile as _tile
from concourse import mybir
from concourse.bass_utils import run_bass_kernel_spmd
from concourse.vector_clock import ScopedClock

F32 = mybir.dt.float32
BF16 = mybir.dt.bfloat16
N, D, K = 262144, 128, 64
NCORES = 8
NSHARD = N // NCORES          # 32768 rows per core
BLK = 1024                    # rows per block
NBLK = NSHARD // BLK          # 32 blocks
NCHUNK = 16                   # DMA chunks (2 blocks each)
RPP = NSHARD // 128           # rows per partition (256)
EPS = 1e-16
LAMB = 1.0


# ---- workarounds: this walrus build rejects >1 sync wait per instruction ----

def _split_multiwait(json_bytes: bytes) -> bytes:
    data = json.loads(json_bytes)
    counter = [0]
    for fn in data["functions"]:
        for blk in fn["blocks"]:
            new_insts = []
            for inst in blk["instructions"]:
                si = inst.get("sync_info")
                waits = (si or {}).get("on_wait") or []
                if len(waits) > 1:
                    for w in waits[:-1]:
                        counter[0] += 1
                        new_insts.append({
                            "debug": inst.get("debug"),
                            "engine": inst["engine"],
                            "ins": [],
                            "name": f"splitw_{counter[0]}_{inst['name']}",
                            "opcode": "EventSemaphore",
                            "outs": [],
                            "sync_info": {"on_update": [], "on_wait": [w]},
                        })
                    si["on_wait"] = [waits[-1]]
                new_insts.append(inst)
            blk["instructions"] = new_insts
    return json.dumps(data).encode()


class PatchedBass(_bass.Bass):
    def to_json_bytes(self) -> bytes:
        return _split_multiwait(super().to_json_bytes())


class SplitDrainTileContext(_tile.TileContext):
    def _drain_and_barrier(self, tick_clock, wait_clock):
        drain_inst = self.nc.sync.drain()
        wait_clock.add_sem_waits(
            drain_inst.ins, ScopedClock({None: tick_clock.global_clock})
        )
        si = drain_inst.ins.sync_info
        if si is not None and len(si.on_wait) > 1:
            waits = list(si.on_wait)
            si.on_wait = waits[:1]
            drain_inst.ins.sync_info = si
            for w in waits[1:]:
                d2 = self.nc.sync.drain()
                si2 = d2.ins.sync_info
                if si2 is None:
                    import copy
                    si2 = copy.copy(si)
                si2.on_wait = [w]
                si2.on_update = []
                d2.ins.sync_info = si2
        self.nc.all_engine_barrier()
        assert self.sems is not None
        popped = self.nc._tile_sem_poison_stack.pop()
        assert popped is self._sem_poison
        self.nc.clear_and_free_semaphores(list(self.sems.allocated().values()))
        self.nc.all_engine_barrier()


# ------------------------------ kernel build ------------------------------

_CACHE = {}


def _build():
    if "nc" in _CACHE:
        return _CACHE["nc"]
    nc = PatchedBass("TRN2", target_bir_lowering=False, debug=False)
    xs_ap = nc.dram_tensor("xs", [NSHARD, D], F32, kind="ExternalInput").ap()
    m2t_ap = nc.dram_tensor("m2t", [D, K], BF16, kind="ExternalInput").ap()
    ind_ap = nc.dram_tensor("ind", [128, 8, 8], BF16, kind="ExternalInput").ap()
    id_ap = nc.dram_tensor("ident", [128, 128], BF16, kind="ExternalInput").ap()
    out_ap = nc.dram_tensor("out", [40, 8, 512], BF16,
                            kind="ExternalOutput").ap()

    Exp = mybir.ActivationFunctionType.Exp
    Ln = mybir.ActivationFunctionType.Ln
    MUL = mybir.AluOpType.mult

    # row = p*(NBLK*8) + b*8 + r: partition p owns a contiguous row range,
    # so each per-block load is one contiguous 4 KiB read per partition.
    xsv = xs_ap.rearrange("(p b r) d -> b p r d", p=128, b=NBLK)

    with SplitDrainTileContext(nc) as tc:
        with tc.tile_pool(name="const", bufs=1) as const, \
             tc.tile_pool(name="xin", bufs=8) as xin, \
             tc.tile_pool(name="xtp", bufs=4) as xtp, \
             tc.tile_pool(name="ep", bufs=5) as ep, \
             tc.tile_pool(name="pp", bufs=4) as pp, \
             tc.tile_pool(name="stage", bufs=1) as stage, \
             tc.tile_pool(name="psT", bufs=2, space="PSUM") as psTp, \
             tc.tile_pool(name="psZ", bufs=2, space="PSUM") as psZp, \
             tc.tile_pool(name="psSW", bufs=2, space="PSUM") as psSWp:

            m2t = const.tile([D, K], BF16)
            nc.sync.dma_start(out=m2t[:], in_=m2t_ap[:])
            ind = const.tile([128, 8, 8], BF16)
            nc.sync.dma_start(out=ind[:], in_=ind_ap[:])
            ident = const.tile([128, 128], BF16)
            nc.sync.dma_start(out=ident[:], in_=id_ap[:])

            # staged S/W per 4-block group (rows 0:8 = S, 32:40 = W,
            # rows 8:32 are dead padding so one evict instruction covers
            # both at the same per-partition free-dim cost)
            stats_sw = stage.tile([40, 8, 512], BF16)
            xc_t = {}     # block -> xin tile
            xT_t = {}     # block -> xtp tile (sbuf xT)
            psZ_t = {}    # pair -> psum z tile
            E_t = {}      # pair -> E tile
            P_t = {}      # pair -> P tile
            psSW_t = {}   # group -> psum stats tile

            for i in range(NBLK + 5):
                # ---- GpSimd: load block i (prefetch governed by pool) ----
                if i < NBLK:
                    xc = xin.tile([128, 8, D], BF16)
                    xc_t[i] = xc
                    nc.gpsimd.dma_start(out=xc[:], in_=xsv[i])

                # ---- PE: z-matmuls for block i-2 (xT evicted 2 iters ago).
                # Even/odd blocks share a 2-bank psZ pair tile so the exp
                # and z*E passes amortize their fixed costs over 2 blocks.
                if 2 <= i <= NBLK + 1:
                    b = i - 2
                    q, h = divmod(b, 2)
                    xTf = xT_t[b][:].rearrange("d r p -> d (r p)")
                    if h == 0:
                        psZ_t[q] = psZp.tile([128, 2, 512], F32, name="psZ")
                    psZ = psZ_t[q]
                    nc.tensor.matmul(psZ[0:64, h, :], m2t[:], xTf[:, 0:512],
                                     start=True, stop=True,
                                     tile_position=(0, 0))
                    nc.tensor.matmul(psZ[64:128, h, :], m2t[:],
                                     xTf[:, 512:1024],
                                     start=True, stop=True,
                                     tile_position=(0, 64))

                # ---- ACT: exp for block pair (i-3, i-2) ----
                if 2 <= i <= NBLK + 1 and (i - 2) % 2 == 1:
                    q = (i - 2) // 2
                    E = ep.tile([128, 2, 512], BF16)
                    E_t[q] = E
                    nc.scalar.activation(E[:], psZ_t[q][:], Exp)

                # ---- DVE: P = z*E for block pair (i-4, i-3) ----
                if 3 <= i <= NBLK + 2 and (i - 3) % 2 == 1:
                    q3 = (i - 3) // 2
                    P = pp.tile([128, 2, 512], BF16)
                    P_t[q3] = P
                    nc.vector.scalar_tensor_tensor(P[:], psZ_t[q3][:], 1.0,
                                                   E_t[q3][:], MUL, MUL)

                # ---- PE: 8 transposes for block i ----
                if i < NBLK:
                    xc = xc_t[i]
                    psT = psTp.tile([128, 8, 128], BF16)
                    for r in range(8):
                        nc.tensor.transpose(psT[:, r, :], xc[:, r, :],
                                            ident[:])
                    xT = xtp.tile([128, 8, 128], BF16)
                    xT_t[i] = xT
                    # all 8 tiles on DVE: its bf16 2x copy beats splitting
                    # (ACT's ~300ns per-instruction fixed cost dominates)
                    nc.vector.tensor_copy(xT[:], psT[:])

                # ---- PE: reduce matmuls for block i-5 (S || W col groups) ----
                if 5 <= i <= NBLK + 4:
                    b5 = i - 5
                    g, j = divmod(b5, 4)
                    if j == 0:
                        psSW_t[g] = psSWp.tile([40, 512], F32, name="psSW")
                    psSW = psSW_t[g]
                    # S group occupies array cols 0:8 -> psum rows 0:8,
                    # W group cols 32:40 -> rows 32:40; distinct col groups
                    # run concurrently on the PE. j==0 matmuls write each
                    # region fully (zeros outside their 2 rows), so later
                    # start=False matmuls accumulate onto clean zeros.
                    q5, h5 = divmod(b5, 2)
                    nc.tensor.matmul(psSW[0:8, :], ind[:, j, :],
                                     E_t[q5][:, h5, :],
                                     start=(j == 0), stop=(j == 3),
                                     tile_position=(0, 0),
                                     skip_group_check=True)
                    nc.tensor.matmul(psSW[32:40, :], ind[:, 4 + j, :],
                                     P_t[q5][:, h5, :],
                                     start=(j == 0), stop=(j == 3),
                                     tile_position=(0, 32),
                                     skip_group_check=True)
                    if j == 3:
                        nc.scalar.copy(stats_sw[:, g, :], psSW[:])
                        # ship this group's raw S/W stats to DRAM now; the
                        # tiny ln/divide final runs on the host in f64
                        nc.sync.dma_start(out=out_ap[:, g, :],
                                          in_=stats_sw[:, g, :])

                # free refs we no longer need (python-side bookkeeping only)
                if i >= 8:
                    xT_t.pop(i - 8, None)
                    qold = (i - 8) // 2
                    psZ_t.pop(qold, None)
                    E_t.pop(qold, None)
                    P_t.pop(qold, None)


    _CACHE["nc"] = nc
    return nc


def _entropy_np(p):
    p = np.where(p <= 0, EPS, p)
    p = np.where(p >= 1, 1.0 - EPS, p)
    return -np.sum(p * np.log(p), axis=-1)


def kernel(x, m):
    nc = _build()

    m2t = (2.0 * np.float64(m).T).astype(ml_dtypes.bfloat16)   # [128, 64]
    ident = np.eye(128, dtype=ml_dtypes.bfloat16)
    ind = np.zeros((128, 8, 8), dtype=ml_dtypes.bfloat16)
    for j in range(4):
        ind[0:64, j, 2 * j] = 1          # S, chunk A (psum rows 0:8)
        ind[64:128, j, 2 * j + 1] = 1    # S, chunk B
        ind[0:64, 4 + j, 2 * j] = 1      # W, chunk A (psum rows 32:40)
        ind[64:128, 4 + j, 2 * j + 1] = 1

    in_maps = []
    for c in range(NCORES):
        in_maps.append({
            "xs": np.ascontiguousarray(x[c * NSHARD:(c + 1) * NSHARD]),
            "m2t": m2t, "ind": ind, "ident": ident,
        })
    _CACHE["last_in_maps"] = in_maps
    res = run_bass_kernel_spmd(nc, in_maps, core_ids=list(range(NCORES)))

    tot_ls = 0.0
    tot_ws = 0.0
    for c in range(NCORES):
        o = np.float64(res.results[c]["out"])   # [40, 8, 512] staged S/W
        S = o[0:8]
        W = o[32:40]
        tot_ls += np.log(S).sum()
        tot_ws += (W / S).sum()
    intra = (tot_ls - tot_ws) / N

    # inter term on host (tiny), replicating the reference exactly
    m64 = np.float64(m)
    mu = m64.mean(axis=0)
    d2 = ((mu[None, :] - m64) ** 2).sum(axis=1)
    zl = -d2
    zl -= zl.max()
    e = np.exp(zl)
    p = e / e.sum()
    inter = _entropy_np(p)

    total = intra - LAMB * inter
    return (np.float32(total), np.float32(intra), np.float32(inter))
